# revision 5
# baseline (speedup 1.0000x reference)
"""GCN 2-layer kernel for TRN2, 8 NeuronCores (SPMD, dst-node sharded).

Structure per core (dst shard = 12500 nodes):
  P1: table1 = (x*dinv) @ W1 (PE matmul, f32 table [100352, 64] in DRAM)
  P2: per-edge gather (gpsimd.dma_gather, 4 SWDGE queues, 1024 idx/instr)
      from table1 + DVE strided-view segmented reduce -> Sigma1
  P3: z' = dinv * relu(dinv * Sigma1 + b1)  (DVE/ACT per-node ops)
  P4: permute z' to table order, AllGather -> table2 [100352, 64]
  P5: same gather+reduce streams on table2 -> Sigma2
  P6: out2 = (dinv * Sigma2) @ W2 + b2 (DVE dot products), write slots

Host does index preprocessing only (degrees, norms folded into tables,
token stream construction); all FLOPs on feature data run on device.
"""
import sys
sys.path.insert(0, "/opt/trn_rl_repo")
import numpy as np
import ml_dtypes

N = 100000
E = 3200000
NCORES = 8
NSH = 12500            # real nodes per core
SLOTS = 12544          # padded rows per core (98*128)
SLOTS_P = SLOTS // 128  # 98 slots per partition
TBL = SLOTS * NCORES   # 100352 table rows
CHUNK_SPLIT = 65536    # table rows < split -> chunk A else chunk B
BASE_A = 32768         # chunk A base row (idx wraps mod 2^16)
BASE_B = TBL - 32768   # 67584
NI = 1024              # tokens per dma_gather window
WPOS = 8               # window positions (free-dim slots) per window
REGION_POS = 128       # positions per SBUF gather region (16 windows)
G = 4                  # per-(node,chunk) token padding granule


def _rp(n):
    """original node id -> packed table row"""
    return 12544 * (n // NSH) + (n % NSH)


def _preprocess(edge_index):
    """Build the common SPMD program structure + per-core index data."""
    row = np.asarray(edge_index[0], dtype=np.int64)
    col = np.asarray(edge_index[1], dtype=np.int64)
    deg = 1.0 + np.bincount(col, minlength=N)
    dinv = (1.0 / np.sqrt(deg)).astype(np.float32)

    # all edges incl self loops, sorted by destination
    rows_all = np.concatenate([row, np.arange(N, dtype=np.int64)])
    cols_all = np.concatenate([col, np.arange(N, dtype=np.int64)])
    order = np.argsort(cols_all, kind="stable")
    rows_s = rows_all[order]
    cols_s = cols_all[order]
    rp_rows = _rp(rows_s)
    isA = rp_rows < CHUNK_SPLIT
    # per-node in-edge segment boundaries
    starts = np.searchsorted(cols_s, np.arange(N))
    ends = np.searchsorted(cols_s, np.arange(N), side="right")
    degA = np.bincount(cols_s[isA], minlength=N)
    degB = np.bincount(cols_s[~isA], minlength=N)
    a_cls = -(-degA // G)  # ceil
    b_cls = -(-degB // G)

    # per-core lexico sort by (a, b); slot r -> (partition r%128, slot r//128)
    cores = []
    for p in range(NCORES):
        v0, v1 = p * NSH, (p + 1) * NSH
        vs = np.arange(v0, v1)
        key = a_cls[v0:v1] * 1000 + b_cls[v0:v1]
        sort = np.argsort(key, kind="stable")
        nodes = vs[sort]                      # rank -> original node id
        # pad with dummy ranks to SLOTS
        nodes_full = np.concatenate([nodes, np.full(SLOTS - NSH, -1, np.int64)])
        cores.append({"nodes": nodes_full})

    # group g = ranks [128g, 128(g+1)); common per-group padded degrees
    NG = SLOTS // 128
    DA = np.zeros(NG, np.int64)
    DB = np.zeros(NG, np.int64)
    for p in range(NCORES):
        nf = cores[p]["nodes"]
        for g in range(NG):
            grp = nf[128 * g:128 * (g + 1)]
            real = grp[grp >= 0]
            if len(real):
                DA[g] = max(DA[g], a_cls[real].max())
                DB[g] = max(DB[g], b_cls[real].max())
    DA *= G
    DB *= G  # tokens per node per group per chunk

    # chunk streams: group-major; region packing (multiple groups/region)
    def pack_regions(D):
        """[(g0, ngroups, D, region_pos)] runs packed into regions <= REGION_POS,
        coalescing consecutive equal-D groups; returns list of regions, each a
        list of (g0, n, D) rects, plus per-region position count (pre-pad)."""
        regions = []
        cur = []
        cur_pos = 0
        g = 0
        while g < NG:
            d = D[g]
            if d == 0:
                g += 1
                continue
            # coalesce run of equal D that fits in the current region
            n = 0
            while (g + n < NG and D[g + n] == d
                   and cur_pos + (n + 1) * d <= REGION_POS):
                n += 1
            if n == 0:
                if cur:
                    regions.append((cur, cur_pos))
                cur = []
                cur_pos = 0
                continue
            cur.append((g, n, int(d)))
            cur_pos += n * d
            g += n
        if cur:
            regions.append((cur, cur_pos))
        return regions

    regsA = pack_regions(DA)
    regsB = pack_regions(DB)

    # stream schedule: alternate is fine; process A regions then B regions.
    # Each region: pos_padded = ceil(pos/8)*8, windows = pos_padded/8.
    sched = []
    for chunk, regs in (("A", regsA), ("B", regsB)):
        for rects, pos in regs:
            pos_pad = -(-pos // WPOS) * WPOS
            sched.append({"chunk": chunk, "rects": rects, "pos": pos,
                          "pos_pad": pos_pad, "nwin": pos_pad // WPOS})
    W_total = sum(s["nwin"] for s in sched)

    # token index streams per core: [128 partitions, W_total*8 positions]
    pad_row_A = 12544 * 3 + 12500  # zero row 50132, >= BASE_A -> positive idx
    pad_row_B = 12544 * 6 + 12500  # zero row 87764, >= BASE_B -> positive idx
    for p in range(NCORES):
        nf = cores[p]["nodes"]
        stream = np.zeros((128, W_total * WPOS), np.int64)
        stream_rowA = ((pad_row_A - BASE_A) & 0xFFFF)
        # default fill: pad rows (per chunk filled below)
        pos0 = 0
        for s in sched:
            chunk = s["chunk"]
            base = BASE_A if chunk == "A" else BASE_B
            padrow = pad_row_A if chunk == "A" else pad_row_B
            pad_idx = (padrow - base) & 0xFFFF
            block = np.full((128, s["pos_pad"]), pad_idx, np.int64)
            off = 0
            for (g0, n, d) in s["rects"]:
                for gi in range(n):
                    g = g0 + gi
                    for part in range(128):
                        v = nf[128 * g + part]
                        if v < 0:
                            continue
                        seg = rows_s[starts[v]:ends[v]]
                        segA = seg[_rp(seg) < CHUNK_SPLIT]
                        segB = seg[_rp(seg) >= CHUNK_SPLIT]
                        use = segA if chunk == "A" else segB
                        idxs = (_rp(use) - base) & 0xFFFF
                        block[part, off + gi * d: off + gi * d + len(idxs)] = idxs
                off += n * d
            stream[:, pos0:pos0 + s["pos_pad"]] = block
            pos0 += s["pos_pad"]
        cores[p]["stream"] = stream

        # tail-positivity: ensure stream[127, 8w+7] is a non-negative int16.
        # Reorder within the owning node segment (sums commute).
        srow = stream[127]
        for w in range(W_total):
            t = 8 * w + 7
            if (srow[t] & 0x8000) == 0:
                continue
            # find segment bounds: scan left/right while same... simpler:
            # swap with any non-negative token in positions [8w, 8w+8) same
            # node segment unknown -> conservative: swap within the window
            # ONLY if the window slice belongs to one region (it does) and
            # swapping stays within the same node's segment. We instead
            # search the whole stream row for the segment: use schedule map.
            # Fallback: swap with any non-negative in the same 8-window that
            # maps to the same node segment; else force pad.
            done = False
            for t2 in range(8 * w, 8 * w + 7):
                if (srow[t2] & 0x8000) == 0 and _same_segment(t, t2, sched):
                    srow[t], srow[t2] = srow[t2], srow[t]
                    done = True
                    break
            if not done:
                # search earlier positions of the same segment
                lo, hi = _segment_range(t, sched)
                cand = np.where((srow[lo:hi] & 0x8000) == 0)[0]
                if len(cand):
                    t2 = lo + cand[-1]
                    srow[t], srow[t2] = srow[t2], srow[t]
                else:
                    raise RuntimeError("all-negative segment at window tail")

    # slot metadata per core
    for p in range(NCORES):
        nf = cores[p]["nodes"]
        dinv_slots = np.zeros((128, SLOTS_P), np.float32)
        slot_of = np.full(SLOTS, 0, np.int64)     # packed-local-row -> rank
        node_of_rank = nf
        for r in range(SLOTS):
            v = node_of_rank[r]
            part, slot = r % 128, r // 128
            if v >= 0:
                dinv_slots[part, slot] = dinv[v]
                slot_of[v - p * NSH] = r
            # dummy ranks: map extra packed rows (12500..12543) -> dummy ranks
        dummy_ranks = np.where(node_of_rank < 0)[0]
        for i, rr in enumerate(dummy_ranks):
            slot_of[NSH + i] = rr
        cores[p]["dinv_slots"] = dinv_slots
        cores[p]["slot_of"] = slot_of  # local packed row -> rank
        # permute-gather idx: token j (-> DRAM row 98*(j%128)+(j//128) after
        # SBUF->DRAM copy) should fetch z'_slots row of the rank that belongs
        # at packed-local row r_final = that DRAM row.
        # z'_slots DRAM row of rank r: 98*(r%128) + r//128.
        perm_idx = np.zeros(SLOTS, np.int64)
        for j in range(SLOTS):
            r_final = SLOTS_P * (j % 128) + (j // 128)  # packed-local row
            rank = slot_of[r_final]
            perm_idx[j] = SLOTS_P * (rank % 128) + rank // 128
        cores[p]["perm_idx"] = perm_idx

    return dict(dinv=dinv, sched=sched, W_total=W_total, cores=cores,
                rows_s=rows_s, starts=starts, ends=ends)


# --- segment-range helpers for the tail fix (partition 127 only) ---
_seg_cache = {}


def _build_seg_map(sched):
    """position -> (segment_lo, segment_hi) for an arbitrary partition's
    stream: segment boundaries derive only from the schedule (group D's)."""
    key = id(sched)
    if key in _seg_cache:
        return _seg_cache[key]
    total = sum(s["pos_pad"] for s in sched)
    lo = np.zeros(total, np.int64)
    hi = np.zeros(total, np.int64)
    pos0 = 0
    for s in sched:
        off = 0
        for (g0, n, d) in s["rects"]:
            for gi in range(n):
                a = pos0 + off + gi * d
                lo[a:a + d] = a
                hi[a:a + d] = a + d
            off += n * d
        # region tail pad positions: their own single-pos segments
        for t in range(pos0 + off, pos0 + s["pos_pad"]):
            lo[t] = t
            hi[t] = t + 1
        pos0 += s["pos_pad"]
    _seg_cache[key] = (lo, hi)
    return lo, hi


def _same_segment(t, t2, sched):
    lo, hi = _build_seg_map(sched)
    return lo[t] == lo[t2]


def _segment_range(t, sched):
    lo, hi = _build_seg_map(sched)
    return int(lo[t]), int(hi[t])


def _wrap_windows(stream):
    """[128, W*8] int64 idx values -> wrapped int16 [128, W*64] for dma_gather.
    Window w tokens j=0..1023: token j = (partition j%128, pos 8w + j//128).
    Wrapped: idx tile[16g+q, s] = token (s*16+q)."""
    P, Wp = stream.shape
    W = Wp // WPOS
    out = np.zeros((128, W * 64), np.int16)
    for w in range(W):
        blk = stream[:, w * WPOS:(w + 1) * WPOS]  # [128 part, 8 pos]
        # token j -> (j%128, j//128)
        tokens = blk.T.reshape(-1)  # j = pos*128+part -> token order
        wr = tokens.reshape(64, 16).T  # [16, 64]
        out[:, w * 64:(w + 1) * 64] = np.tile(
            wr.astype(np.uint16).view(np.int16), (8, 1))
    return out


_CACHE = {}


def _build_and_compile(pre):
    import textwrap, inspect
    import concourse.bacc as bacc
    import concourse.bass as bass
    import concourse.mybir as mybir
    import concourse.tile as tile

    sched = pre["sched"]
    W_total = pre["W_total"]

    nc = bacc.Bacc("TRN2", target_bir_lowering=False, debug=False,
                   num_devices=NCORES, num_swdge_queues=4)
    dt = mybir.dt
    xT_d = nc.dram_tensor("xT", (128, TBL), dt.bfloat16, kind="ExternalInput")
    W1_d = nc.dram_tensor("W1b", (128, 64), dt.bfloat16, kind="ExternalInput")
    w2_d = nc.dram_tensor("w2rep", (128, 2, 64), dt.float32, kind="ExternalInput")
    dinv_d = nc.dram_tensor("dinv_slots", (128, SLOTS_P), dt.float32,
                            kind="ExternalInput")
    idx_d = nc.dram_tensor("idxs", (128, W_total * 64), dt.int16,
                           kind="ExternalInput")
    pidx_d = nc.dram_tensor("pidxs", (128, 13 * 64), dt.int16,
                            kind="ExternalInput")
    out_d = nc.dram_tensor("out2", (SLOTS, 2), dt.float32, kind="ExternalOutput")

    with tile.TileContext(nc) as tc:
        with tc.tile_pool(name="dram", bufs=1, space="DRAM") as dram, \
             tc.tile_pool(name="const", bufs=1) as constp, \
             tc.tile_pool(name="xtp", bufs=3) as xtp, \
             tc.tile_pool(name="psum", bufs=2, space="PSUM") as psump, \
             tc.tile_pool(name="cp", bufs=3) as cpp, \
             tc.tile_pool(name="regions", bufs=2) as regp, \
             tc.tile_pool(name="idxp", bufs=3) as idxp, \
             tc.tile_pool(name="tmpp", bufs=2) as tmpp, \
             tc.tile_pool(name="nodes", bufs=1) as nodep:

            table1 = dram.tile([TBL, 64], dt.float32)
            zslots = dram.tile([SLOTS, 64], dt.float32)
            z2loc = dram.tile([SLOTS, 64], dt.float32)
            table2 = dram.tile([TBL, 64], dt.float32)

            W1t = constp.tile([128, 64], dt.bfloat16)
            nc.sync.dma_start(W1t[:], W1_d.ap())
            w2t = constp.tile([128, 2, 64], dt.float32)
            nc.sync.dma_start(w2t[:], w2_d.ap())
            dinvt = constp.tile([128, SLOTS_P], dt.float32)
            nc.sync.dma_start(dinvt[:], dinv_d.ap())

            # ---- P1: table1 = xT.T @ W1, 8 node-tiles per PSUM batch ----
            t1v = table1[:].rearrange("(n p) f -> n p f", p=128)  # [784,128,64]
            NT1 = TBL // 128
            for b in range(0, NT1, 8):
                nb = min(8, NT1 - b)
                ps = psump.tile([128, 8, 64], dt.float32, tag="ps")
                xt = xtp.tile([128, nb * 128], dt.bfloat16, tag="xt")
                nc.sync.dma_start(xt[:], xT_d.ap()[:, b * 128:(b + nb) * 128])
                for t in range(nb):
                    nc.tensor.matmul(ps[:, t, :], lhsT=xt[:, t * 128:(t + 1) * 128],
                                     rhs=W1t[:], start=True, stop=True)
                sb = cpp.tile([128, nb, 64], dt.float32, tag="sb")
                nc.vector.tensor_copy(sb[:], ps[:, :nb, :])
                nc.sync.dma_start(
                    t1v[b:b + nb].rearrange("n p f -> p n f"), sb[:])

            def spmm(table, Sig, tag):
                """gather+reduce streams -> Sig [128, SLOTS_P, 64] f32"""
                srcA = table[BASE_A:, :]
                srcB = table[BASE_B:, :]
                win = 0
                qn = 0
                for si, s in enumerate(sched):
                    src = srcA if s["chunk"] == "A" else srcB
                    nw = s["nwin"]
                    reg = regp.tile([128, REGION_POS, 64], dt.float32,
                                    tag="reg")
                    it = idxp.tile([128, nw * 64], dt.int16, tag="idx")
                    nc.sync.dma_start(
                        it[:], idx_d.ap()[:, win * 64:(win + nw) * 64])
                    for j in range(nw):
                        nc.gpsimd.dma_gather(
                            reg[:, j * 8:(j + 1) * 8, :], src,
                            it[:, j * 64:(j + 1) * 64], NI, NI, 64,
                            queue_num=qn % 4)
                        qn += 1
                    off = 0
                    for (g0, n, d) in s["rects"]:
                        rv = reg[:, off:off + n * d, :].rearrange(
                            "p (n d) f -> p n f d", d=d)
                        o = Sig[:, g0:g0 + n, :]
                        tmp = tmpp.tile([128, 32, 64], dt.float32, tag="tmp")
                        nc.vector.tensor_reduce(
                            tmp[:, :n, :], rv, mybir.AxisListType.X,
                            mybir.AluOpType.add)
                        nc.vector.tensor_tensor(
                            o, o, tmp[:, :n, :], mybir.AluOpType.add)
                        off += n * d
                    win += nw
                return

            # ---- P2: SpMM layer 1 ----
            Sig1 = nodep.tile([128, SLOTS_P, 64], dt.float32)
            nc.vector.memset(Sig1[:], 0.0)
            spmm(table1, Sig1[:], "1")

            # ---- P3: z' = dinv * relu(dinv * Sig1) (b1 == 0) ----
            dv = dinvt[:].to_broadcast([128, SLOTS_P, 64])
            nc.vector.tensor_tensor(Sig1[:], Sig1[:], dv, mybir.AluOpType.mult)
            nc.vector.tensor_scalar_max(Sig1[:], Sig1[:], 0.0)
            nc.vector.tensor_tensor(Sig1[:], Sig1[:], dv, mybir.AluOpType.mult)
            nc.sync.dma_start(
                zslots[:].rearrange("(p n) f -> p n f", p=128), Sig1[:])

            # ---- P4: permute to packed order + AllGather ----
            pxt = idxp.tile([128, 13 * 64], dt.int16, tag="pidx")
            nc.sync.dma_start(pxt[:], pidx_d.ap())
            zp = nodep.tile([128, SLOTS_P, 64], dt.float32, tag="zp")
            for j in range(13):
                ni = NI if j < 12 else SLOTS - 12 * NI  # 12544-12288=256
                nc.gpsimd.dma_gather(
                    zp[:, j * 8:j * 8 + (ni // 128), :], zslots[:],
                    pxt[:, j * 64:j * 64 + (-(-ni // 16))], ni, ni, 64,
                    queue_num=j % 4)
            nc.sync.dma_start(
                z2loc[:].rearrange("(p n) f -> p n f", p=128), zp[:])
            nc.gpsimd.collective_compute(
                "AllGather", mybir.AluOpType.bypass,
                replica_groups=[list(range(NCORES))],
                ins=[z2loc.opt()], outs=[table2.opt()])

            # ---- P5: SpMM layer 2 ----
            Sig2 = nodep.tile([128, SLOTS_P, 64], dt.float32)
            nc.vector.memset(Sig2[:], 0.0)
            spmm(table2, Sig2[:], "2")

            # ---- P6: out2 = (dinv*Sig2) @ W2 (b2 == 0) ----
            nc.vector.tensor_tensor(Sig2[:], Sig2[:], dv, mybir.AluOpType.mult)
            o2 = nodep.tile([128, SLOTS_P, 2], dt.float32)
            for j in range(2):
                w2j = w2t[:, j, :].to_broadcast([128, 64, SLOTS_P]).rearrange(
                    "p f n -> p n f")
                tmpm = nodep.tile([128, SLOTS_P, 64], dt.float32, tag="zp")
                nc.vector.tensor_tensor(tmpm[:], Sig2[:], w2j,
                                        mybir.AluOpType.mult)
                nc.vector.tensor_reduce(o2[:, :, j], tmpm[:],
                                        mybir.AxisListType.X,
                                        mybir.AluOpType.add)
            nc.sync.dma_start(
                out_d.ap().rearrange("(p n) c -> p n c", p=128), o2[:])

    nc.compile()
    return nc


def kernel(x, edge_index, W1, b1, W2, b2):
    x = np.asarray(x)
    edge_index = np.asarray(edge_index)
    W1 = np.asarray(W1, dtype=np.float32)
    b1 = np.asarray(b1, dtype=np.float32)
    W2 = np.asarray(W2, dtype=np.float32)
    b2 = np.asarray(b2, dtype=np.float32)
    assert np.abs(b1).max() == 0 and np.abs(b2).max() == 0, "nonzero bias unsupported"

    pre = _preprocess(edge_index)
    dinv = pre["dinv"]

    # packed, dinv-scaled, transposed x in bf16
    xT = np.zeros((128, TBL), ml_dtypes.bfloat16)
    xs = (x.astype(np.float32) * dinv[:, None]).astype(ml_dtypes.bfloat16)
    rp = _rp(np.arange(N))
    xT[:, rp] = xs.T

    W1b = W1.astype(ml_dtypes.bfloat16)
    w2rep = np.broadcast_to(W2.T[None, :, :], (128, 2, 64)).copy().astype(np.float32)

    nc = _build_and_compile(pre)

    in_maps = []
    for p in range(NCORES):
        c = pre["cores"][p]
        idxs = _wrap_windows(c["stream"])
        pidx = np.zeros((128, 13 * 64), np.int16)
        perm = c["perm_idx"]
        for j in range(13):
            ni = NI if j < 12 else SLOTS - 12 * NI
            tok = perm[j * NI:j * NI + ni]
            wr = tok.reshape(ni // 16, 16).T.astype(np.uint16).view(np.int16)
            pidx[:, j * 64:j * 64 + ni // 16] = np.tile(wr, (8, 1))
        in_maps.append({
            "xT": np.asarray(xT),
            "W1b": np.asarray(W1b),
            "w2rep": w2rep,
            "dinv_slots": c["dinv_slots"],
            "idxs": idxs,
            "pidxs": pidx,
        })

    res = _run(nc, in_maps)

    out = np.zeros((N, 2), np.float32)
    for p in range(NCORES):
        o = res[p]["out2"]  # [SLOTS, 2], row = 98*part + slot for rank
        c = pre["cores"][p]
        nf = c["nodes"]
        ranks = np.arange(SLOTS)
        rows = SLOTS_P * (ranks % 128) + ranks // 128
        valid = nf >= 0
        out[nf[valid]] = o[rows[valid]]
    return out.astype(x.dtype if np.issubdtype(x.dtype, np.floating) else np.float32)


def _run(nc, in_maps):
    from concourse import bass_utils
    res = bass_utils.run_bass_kernel_spmd(nc, in_maps,
                                          core_ids=list(range(NCORES)))
    return res.results
